# revision 7
# baseline (speedup 1.0000x reference)
"""Trainium2 kernel for nn_HardNegativeContrastiveLoss.

Math note (exact, not an approximation): the reference masks only the
(i, B+i)/(B+i, i) positive pairs of the similarity matrix but leaves the
diagonal unmasked.  After row-normalization every diagonal entry is
z_r.z_r / T = 1/T, and every off-diagonal entry is cos(z_r, z_j)/T < 1/T,
so hardest_neg[r] == 1/T for every row.  The relu argument
1/T + margin - pos is then always >= 1/T + margin - 1 > 0, hence

    loss = 1/T + margin - mean_i( z1_i . z2_i / (||z1_i|| ||z2_i||) )

This kernel therefore computes the per-row cosine between z1 and z2 —
a pure memory-bound row reduction over the 8 MB of input, sharded
row-wise across the 8 NeuronCores (1024 rows of z1+z2 per core).  Each
core emits its 128 partial sums of cosines; the host combines them.
"""

import os
import sys

import numpy as np

for _p in (
    "/root/.axon_site",
    "/root/.axon_site/_ro/trn_rl_repo",
    "/root/.axon_site/_ro/pypackages",
    "/opt/trn_rl_repo",
):
    if os.path.isdir(_p) and _p not in sys.path:
        sys.path.append(_p)

import concourse.bass as bass
import concourse.mybir as mybir
from concourse import bass_utils

B, D = 8192, 128
N_CORES = 8
ROWS = B // N_CORES  # rows of z1 (and of z2) handled per core
S = ROWS // 128      # row-groups of 128 per core
H = S // 2
TEMPERATURE = 0.1
MARGIN = 0.5

_cache = {}


def _build():
    f32 = mybir.dt.float32
    nc = bass.Bass()
    z1p = nc.declare_dram_parameter("z1c", [ROWS, D], f32, isOutput=False)
    z2p = nc.declare_dram_parameter("z2c", [ROWS, D], f32, isOutput=False)
    outp = nc.declare_dram_parameter("partial", [128, 1], f32, isOutput=True)

    # Partition p holds rows 8p..8p+7, so the per-partition DMA source is
    # one contiguous 4 KB run of HBM.
    z1_ap = z1p[:].rearrange("(p s) d -> p s d", p=128)  # [128, S, D]
    z2_ap = z2p[:].rearrange("(p s) d -> p s d", p=128)

    with (
        nc.sbuf_tensor([128, S * D], f32) as z1t,
        nc.sbuf_tensor([128, S * D], f32) as z2t,
        nc.sbuf_tensor([128, S * D], f32) as z1sq,
        nc.sbuf_tensor([128, S * D], f32) as z2sq,
        nc.sbuf_tensor([128, S * D], f32) as scratch,
        nc.sbuf_tensor([128, S], f32) as dots,
        nc.sbuf_tensor([128, S], f32) as n1,
        nc.sbuf_tensor([128, S], f32) as n2,
        nc.sbuf_tensor([128, S], f32) as nsq,
        nc.sbuf_tensor([128, S], f32) as nrm,
        nc.sbuf_tensor([128, S], f32) as rec,
        nc.sbuf_tensor([128, S], f32) as pos,
        nc.sbuf_tensor([128, 1], f32) as rowsum,
        nc.semaphore("ld0_sem") as ld0_sem,
        nc.semaphore("ld1_sem") as ld1_sem,
        nc.semaphore("st_sem") as st_sem,
        nc.semaphore("act_sem") as act_sem,
        nc.semaphore("dve_sem") as dve_sem,
        nc.semaphore("done_sem") as done_sem,
        nc.Block() as block,
    ):

        @block.sync
        def _(sync):
            sync.dma_start(out=z1t[:, : H * D], in_=z1_ap[:, :H, :]).then_inc(
                ld0_sem, 16
            )
            sync.dma_start(out=z2t[:, : H * D], in_=z2_ap[:, :H, :]).then_inc(
                ld0_sem, 16
            )
            sync.dma_start(out=z1t[:, H * D :], in_=z1_ap[:, H:, :]).then_inc(
                ld1_sem, 16
            )
            sync.dma_start(out=z2t[:, H * D :], in_=z2_ap[:, H:, :]).then_inc(
                ld1_sem, 16
            )
            sync.wait_ge(done_sem, 1)
            sync.dma_start(out=outp[:], in_=rowsum[:, :]).then_inc(st_sem, 16)

        @block.scalar
        def _(scalar):
            # squares (z.z per element); halves so work starts after the
            # first half of the loads
            scalar.wait_ge(ld0_sem, 32)
            nc.scalar.square(z1sq[:, : H * D], z1t[:, : H * D]).then_inc(act_sem, 1)
            nc.scalar.square(z2sq[:, : H * D], z2t[:, : H * D]).then_inc(act_sem, 1)
            scalar.wait_ge(ld1_sem, 32)
            nc.scalar.square(z1sq[:, H * D :], z1t[:, H * D :]).then_inc(act_sem, 1)
            nc.scalar.square(z2sq[:, H * D :], z2t[:, H * D :]).then_inc(act_sem, 1)
            # sqrt of n1*n2 once DVE has produced it (dve_sem counts DVE ops:
            # nsq is the 11th)
            scalar.wait_ge(dve_sem, 11)
            nc.scalar.sqrt(nrm[:, :], nsq[:, :]).then_inc(act_sem, 1)

        @block.vector
        def _(vector):
            # per-group fused multiply+reduce: dots[:, s] = sum_d z1*z2
            # (dve_sem: ops 1..8)
            vector.wait_ge(ld0_sem, 32)
            for s in range(S):
                if s == H:
                    vector.wait_ge(ld1_sem, 32)
                nc.vector.scalar_tensor_tensor(
                    out=scratch[:, s * D : (s + 1) * D],
                    in0=z1t[:, s * D : (s + 1) * D],
                    scalar=1.0,
                    in1=z2t[:, s * D : (s + 1) * D],
                    op0=mybir.AluOpType.mult,
                    op1=mybir.AluOpType.mult,
                    accum_out=dots[:, s : s + 1],
                ).then_inc(dve_sem, 1)
            # segmented row-reduces of the squares (ops 9, 10)
            vector.wait_ge(act_sem, 3)
            nc.vector.reduce_sum(
                n1[:, :],
                z1sq[:, :].rearrange("p (s d) -> p s d", d=D),
                axis=mybir.AxisListType.X,
            ).then_inc(dve_sem, 1)
            vector.wait_ge(act_sem, 4)
            nc.vector.reduce_sum(
                n2[:, :],
                z2sq[:, :].rearrange("p (s d) -> p s d", d=D),
                axis=mybir.AxisListType.X,
            ).then_inc(dve_sem, 1)
            # nsq = n1 * n2 (op 11)
            vector.wait_ge(dve_sem, 10)
            nc.vector.tensor_mul(nsq[:, :], n1[:, :], n2[:, :]).then_inc(dve_sem, 1)
            # pos = dots / sqrt(n1*n2); rowsum = sum_s pos (ops 12, 13, 14)
            vector.wait_ge(act_sem, 5)
            nc.vector.reciprocal(rec[:, :], nrm[:, :]).then_inc(dve_sem, 1)
            vector.wait_ge(dve_sem, 12)
            nc.vector.tensor_mul(pos[:, :], dots[:, :], rec[:, :]).then_inc(dve_sem, 1)
            vector.wait_ge(dve_sem, 13)
            nc.vector.reduce_sum(
                rowsum[:, :], pos[:, :], axis=mybir.AxisListType.X
            ).then_inc(done_sem, 1)

    return nc


def kernel(z1: np.ndarray, z2: np.ndarray) -> np.ndarray:
    z1 = np.ascontiguousarray(np.asarray(z1, dtype=np.float32))
    z2 = np.ascontiguousarray(np.asarray(z2, dtype=np.float32))
    assert z1.shape == (B, D) and z2.shape == (B, D)

    if "nc" not in _cache:
        _cache["nc"] = _build()
    nc = _cache["nc"]

    core_ids = list(range(N_CORES))
    in_maps = [
        {
            "z1c": z1[c * ROWS : (c + 1) * ROWS],
            "z2c": z2[c * ROWS : (c + 1) * ROWS],
        }
        for c in core_ids
    ]
    res = bass_utils.run_bass_kernel_spmd(nc, in_maps, core_ids)
    total = np.float64(0.0)
    for c in core_ids:
        total += np.sum(res.results[c]["partial"].astype(np.float64))
    loss = 1.0 / TEMPERATURE + MARGIN - total / float(B)
    return np.asarray(loss, dtype=np.float32)


# revision 15
# speedup vs baseline: 1.0771x; 1.0771x over previous
"""Trainium2 kernel for nn_HardNegativeContrastiveLoss.

Math note (exact, not an approximation): the reference masks only the
(i, B+i)/(B+i, i) positive pairs of the similarity matrix but leaves the
diagonal unmasked.  After row-normalization every diagonal entry is
z_r.z_r / T = 1/T, and every off-diagonal entry is cos(z_r, z_j)/T < 1/T,
so hardest_neg[r] == 1/T for every row.  The relu argument
1/T + margin - pos is then always >= 1/T + margin - 1 > 0, hence

    loss = 1/T + margin - mean_i( z1_i . z2_i / (||z1_i|| ||z2_i||) )

This kernel therefore computes the per-row cosine between z1 and z2 —
a pure memory-bound row reduction over the 8 MB of input, sharded
row-wise across the 8 NeuronCores (1024 rows of z1+z2 per core).  Each
core emits its 128 partial sums of cosines; the host combines them.

Work split per core (raw Bass, manual semaphores):
  SP    : four half loads (HWDGE ring) + result store
  ACT   : warm-up (hides the cold activation-table load), z1 squares,
          final sqrt
  DVE   : per-group z1.z2 dots (fused multiply+reduce), z1 norm
          reduces, final normalize chain
"""

import os
import sys
from contextlib import ExitStack

import numpy as np

for _p in (
    "/root/.axon_site",
    "/root/.axon_site/_ro/trn_rl_repo",
    "/root/.axon_site/_ro/pypackages",
    "/opt/trn_rl_repo",
):
    if os.path.isdir(_p) and _p not in sys.path:
        sys.path.append(_p)

import concourse.bass as bass
import concourse.mybir as mybir
from concourse import bass_utils

B, D = 8192, 128
N_CORES = 8
ROWS = B // N_CORES  # rows of z1 (and of z2) handled per core
S = ROWS // 128      # row-groups of 128 per core
H = S // 2
TEMPERATURE = 0.1
MARGIN = 0.5

_cache = {}


def _build():
    f32 = mybir.dt.float32
    mult = mybir.AluOpType.mult
    X = mybir.AxisListType.X
    nc = bass.Bass()
    z1p = nc.declare_dram_parameter("z1c", [ROWS, D], f32, isOutput=False)
    z2p = nc.declare_dram_parameter("z2c", [ROWS, D], f32, isOutput=False)
    outp = nc.declare_dram_parameter("partial", [128, 1], f32, isOutput=True)

    # Partition p holds rows 8p..8p+7, so the per-partition DMA source is
    # one contiguous 4 KB run of HBM.
    z1_ap = z1p[:].rearrange("(p s) d -> p s d", p=128)  # [128, S, D]
    z2_ap = z2p[:].rearrange("(p s) d -> p s d", p=128)

    with ExitStack() as ctx:
        z1t = ctx.enter_context(nc.sbuf_tensor([128, S * D], f32))
        z2t = ctx.enter_context(nc.sbuf_tensor([128, S * D], f32))
        z1sq = ctx.enter_context(nc.sbuf_tensor([128, S * D], f32))
        vscr = ctx.enter_context(nc.sbuf_tensor([128, S * D], f32))
        gscr = ctx.enter_context(nc.sbuf_tensor([128, S * D], f32))
        dots = ctx.enter_context(nc.sbuf_tensor([128, S], f32))
        n1 = ctx.enter_context(nc.sbuf_tensor([128, S], f32))
        n2 = ctx.enter_context(nc.sbuf_tensor([128, S], f32))
        nsq = ctx.enter_context(nc.sbuf_tensor([128, S], f32))
        nrm = ctx.enter_context(nc.sbuf_tensor([128, S], f32))
        rec = ctx.enter_context(nc.sbuf_tensor([128, S], f32))
        pos = ctx.enter_context(nc.sbuf_tensor([128, S], f32))
        rowsum = ctx.enter_context(nc.sbuf_tensor([128, 1], f32))
        wtile = ctx.enter_context(nc.sbuf_tensor([128, 1], f32))
        z1a_sem = ctx.enter_context(nc.semaphore("z1a_sem"))
        z1b_sem = ctx.enter_context(nc.semaphore("z1b_sem"))
        z2a_sem = ctx.enter_context(nc.semaphore("z2a_sem"))
        z2b_sem = ctx.enter_context(nc.semaphore("z2b_sem"))
        st_sem = ctx.enter_context(nc.semaphore("st_sem"))
        act_sem = ctx.enter_context(nc.semaphore("act_sem"))
        dve_sem = ctx.enter_context(nc.semaphore("dve_sem"))
        done_sem = ctx.enter_context(nc.semaphore("done_sem"))
        block = ctx.enter_context(nc.Block())

        ones = nc.const_aps.scalar_like(1.0, wtile[:, :])

        def dot_group(s):
            return nc.vector.scalar_tensor_tensor(
                out=vscr[:, s * D : (s + 1) * D],
                in0=z1t[:, s * D : (s + 1) * D],
                scalar=1.0,
                in1=z2t[:, s * D : (s + 1) * D],
                op0=mult,
                op1=mult,
                accum_out=dots[:, s : s + 1],
            )

        @block.sync
        def _(sync):
            sync.dma_start(out=z1t[:, : H * D], in_=z1_ap[:, :H, :]).then_inc(
                z1a_sem, 16
            )
            sync.dma_start(out=z2t[:, : H * D], in_=z2_ap[:, :H, :]).then_inc(
                z2a_sem, 16
            )
            sync.dma_start(out=z1t[:, H * D :], in_=z1_ap[:, H:, :]).then_inc(
                z1b_sem, 16
            )
            sync.dma_start(out=z2t[:, H * D :], in_=z2_ap[:, H:, :]).then_inc(
                z2b_sem, 16
            )
            sync.wait_ge(done_sem, 1)
            sync.dma_start(out=outp[:], in_=rowsum[:, :]).then_inc(st_sem, 16)

        @block.scalar
        def _(scalar):
            # a1: warm-up -- pays the cold activation-table load during the
            # DMA window instead of on the critical path
            nc.scalar.square(wtile[:, :], ones).then_inc(act_sem, 1)
            # a2/a3: z1 squares per half
            scalar.wait_ge(z1a_sem, 16)
            nc.scalar.square(z1sq[:, : H * D], z1t[:, : H * D]).then_inc(act_sem, 1)
            scalar.wait_ge(z2a_sem, 16)
            nc.scalar.square(gscr[:, : H * D], z2t[:, : H * D]).then_inc(act_sem, 1)
            scalar.wait_ge(z1b_sem, 16)
            nc.scalar.square(z1sq[:, H * D :], z1t[:, H * D :]).then_inc(act_sem, 1)
            scalar.wait_ge(z2b_sem, 16)
            nc.scalar.square(gscr[:, H * D :], z2t[:, H * D :]).then_inc(act_sem, 1)
            # a6: sqrt(n1*n2) once DVE has produced nsq (dve op 13)
            scalar.wait_ge(dve_sem, 13)
            nc.scalar.sqrt(nrm[:, :], nsq[:, :]).then_inc(act_sem, 1)

        @block.vector
        def _(vector):
            # v1..v4: dots h0 (fused multiply+reduce)
            vector.wait_ge(z1a_sem, 16)
            vector.wait_ge(z2a_sem, 16)
            for s in range(H):
                dot_group(s).then_inc(dve_sem, 1)
            # v5: z1 norm reduce, first half
            vector.wait_ge(act_sem, 2)
            nc.vector.reduce_sum(
                n1[:, :H],
                z1sq[:, : H * D].rearrange("p (s d) -> p s d", d=D),
                axis=X,
            ).then_inc(dve_sem, 1)
            # v6: z2 norm reduce, first half
            vector.wait_ge(act_sem, 3)
            nc.vector.reduce_sum(
                n2[:, :H],
                gscr[:, : H * D].rearrange("p (s d) -> p s d", d=D),
                axis=X,
            ).then_inc(dve_sem, 1)
            # v7..v10: dots h1
            vector.wait_ge(z1b_sem, 16)
            vector.wait_ge(z2b_sem, 16)
            for s in range(H, S):
                dot_group(s).then_inc(dve_sem, 1)
            # v11: z1 norm reduce, second half
            vector.wait_ge(act_sem, 4)
            nc.vector.reduce_sum(
                n1[:, H:],
                z1sq[:, H * D :].rearrange("p (s d) -> p s d", d=D),
                axis=X,
            ).then_inc(dve_sem, 1)
            # v12: z2 norm reduce, second half
            vector.wait_ge(act_sem, 5)
            nc.vector.reduce_sum(
                n2[:, H:],
                gscr[:, H * D :].rearrange("p (s d) -> p s d", d=D),
                axis=X,
            ).then_inc(dve_sem, 1)
            # v13: nsq = n1 * n2
            vector.wait_ge(dve_sem, 12)
            nc.vector.tensor_mul(nsq[:, :], n1[:, :], n2[:, :]).then_inc(dve_sem, 1)
            # v14: rec = 1/sqrt(n1*n2); v15: rowsum = sum_s dots*rec (fused)
            vector.wait_ge(act_sem, 6)
            nc.vector.reciprocal(rec[:, :], nrm[:, :]).then_inc(dve_sem, 1)
            vector.wait_ge(dve_sem, 14)
            nc.vector.scalar_tensor_tensor(
                out=pos[:, :],
                in0=dots[:, :],
                scalar=1.0,
                in1=rec[:, :],
                op0=mult,
                op1=mult,
                accum_out=rowsum[:, :],
            ).then_inc(done_sem, 1)

    return nc


def kernel(z1: np.ndarray, z2: np.ndarray) -> np.ndarray:
    z1 = np.ascontiguousarray(np.asarray(z1, dtype=np.float32))
    z2 = np.ascontiguousarray(np.asarray(z2, dtype=np.float32))
    assert z1.shape == (B, D) and z2.shape == (B, D)

    if "nc" not in _cache:
        _cache["nc"] = _build()
    nc = _cache["nc"]

    core_ids = list(range(N_CORES))
    in_maps = [
        {
            "z1c": z1[c * ROWS : (c + 1) * ROWS],
            "z2c": z2[c * ROWS : (c + 1) * ROWS],
        }
        for c in core_ids
    ]
    res = bass_utils.run_bass_kernel_spmd(nc, in_maps, core_ids)
    total = np.float64(0.0)
    for c in core_ids:
        total += np.sum(res.results[c]["partial"].astype(np.float64))
    loss = 1.0 / TEMPERATURE + MARGIN - total / float(B)
    return np.asarray(loss, dtype=np.float32)
